# revision 19
# baseline (speedup 1.0000x reference)
"""Trainium2 Bass kernel for Bahdanau-style attention (nn_Attention_11742440587335).

reference computation (per batch b):
    att1 = enc[b] @ W_enc + b_enc          # [N, A]
    att2 = hidden[b] @ W_dec + b_dec       # [A]
    att  = relu(att1 + att2)               # [N, A]
    s    = att @ W_full[:, 0] + b_full     # [N]   (b_full dropped: softmax shift-invariant)
    alpha= softmax(s)                      # [N]
    ctx  = alpha @ enc[b]                  # [E]

Sharding: pure data parallel, B=256 split as 32 batches on each of 8 cores.

Per-core dataflow (pairs of batches g = (b0, b1)):
  - enc rows loaded HBM->SBUF fp32 via HWDGE at line rate, then cast to bf16
    on GpSimd/VectorE (SWDGE cast-DMA measured ~6x below line rate); natural
    layout [n, E] as two partition tiles per batch: [128, 2048] + [68, 2048].
  - PE-transpose 128-col chunks into encT [E-part, n-cols] for the projection.
  - projection att1.T accumulated over 16 E-tiles with W_enc stationary.
  - bias+relu fused on ScalarE (per-partition bias = att2.T column, computed
    once on-device from hidden).
  - scores via W_full-stationary matmul; per-pair softmax on VectorE/ScalarE.
  - alpha PE-transposed back to partitions; context = alpha.T @ enc_nat on PE.
"""

import sys

for _p in ("/opt/trn_rl_repo",):
    if _p not in sys.path:
        sys.path.insert(0, _p)

import numpy as np
import ml_dtypes

import concourse.bass as bass
import concourse.bacc as bacc
import concourse.tile as tile
from concourse import mybir
from concourse.bass_utils import run_bass_kernel_spmd

F32 = mybir.dt.float32
BF16 = mybir.dt.bfloat16
AF = mybir.ActivationFunctionType
AX = mybir.AxisListType

B, N, E, A, D = 256, 196, 2048, 512, 512
NCORES = 8
BC = B // NCORES          # 32 batches per core
NPAIRS = BC // 2          # 16
ET = E // 128             # 16 e-tiles
AT = A // 128             # 4 a-tiles
DT = D // 128             # 4 d-tiles
N0, N1 = 128, N - 128     # 128 + 68 row split per batch
N1P = 80                  # r1 partition tile padded to mult-of-16 for xbar
NBP = 208                 # per-batch padded col span in encT (xbar mode)
USE_XBAR = False          # enc transposes via DMA xbar instead of PE


def build_nc():
    nc = bacc.Bacc(None)

    enc_d = nc.declare_dram_parameter("enc", [BC * N, E], F32, isOutput=False)
    hid_d = nc.declare_dram_parameter("hidden_bf", [BC, D], BF16, isOutput=False)
    wenc_d = nc.declare_dram_parameter("w_enc_bf", [E, A], BF16, isOutput=False)
    wdec_d = nc.declare_dram_parameter("w_dec_bf", [D, A], BF16, isOutput=False)
    wfull_d = nc.declare_dram_parameter("w_full_t", [128, AT], BF16, isOutput=False)
    bsum_d = nc.declare_dram_parameter("b_sum_t", [128, AT], F32, isOutput=False)
    ident_d = nc.declare_dram_parameter("ident", [128, 128], BF16, isOutput=False)
    ctx_d = nc.declare_dram_parameter("context", [BC, E], F32, isOutput=True)
    alpha_d = nc.declare_dram_parameter("alpha", [BC * N], F32, isOutput=True)

    with tile.TileContext(nc) as tc, \
         tc.tile_pool(name="singles", bufs=1) as singles, \
         tc.tile_pool(name="r0f", bufs=3) as pool_r0f, \
         tc.tile_pool(name="r1f", bufs=3) as pool_r1f, \
         tc.tile_pool(name="r0", bufs=5) as pool_r0, \
         tc.tile_pool(name="r1", bufs=5) as pool_r1, \
         tc.tile_pool(name="enct", bufs=3) as pool_encT, \
         tc.tile_pool(name="attt", bufs=3) as pool_attT, \
         tc.tile_pool(name="alph", bufs=4) as pool_al, \
         tc.tile_pool(name="at", bufs=4) as pool_aT, \
         tc.tile_pool(name="ctxs", bufs=4) as pool_ctx, \
         tc.tile_pool(name="ps_small", bufs=2 if USE_XBAR else 3, space="PSUM") as ps_small, \
         tc.tile_pool(name="ps_proj", bufs=3 if USE_XBAR else 2, space="PSUM") as ps_proj, \
         tc.tile_pool(name="ps_vec", bufs=1, space="PSUM") as ps_vec, \
         tc.tile_pool(name="ps_ctx", bufs=2, space="PSUM") as ps_ctx:

        # ---- constants / weights ----
        w_enc_sb = singles.tile([128, ET, A], BF16)
        nc.sync.dma_start(out=w_enc_sb, in_=wenc_d.rearrange("(t p) a -> p t a", p=128))
        w_dec_sb = singles.tile([128, DT, A], BF16)
        nc.sync.dma_start(out=w_dec_sb, in_=wdec_d.rearrange("(t p) a -> p t a", p=128))
        w_full_sb = singles.tile([128, AT], BF16)
        nc.sync.dma_start(out=w_full_sb, in_=wfull_d[:, :])
        b_sum_sb = singles.tile([128, AT], F32)
        nc.sync.dma_start(out=b_sum_sb, in_=bsum_d[:, :])
        ident_sb = singles.tile([128, 128], BF16)
        nc.sync.dma_start(out=ident_sb, in_=ident_d[:, :])
        hid_sb = singles.tile([BC, D], BF16)
        nc.sync.dma_start(out=hid_sb, in_=hid_d[:, :])

        # ---- att2.T (+ b_enc + b_dec): bias_sb[a-tile] = [128, BC] f32 ----
        hT_sb = singles.tile([128, DT, BC], BF16)
        for d in range(DT):
            pst = ps_small.tile([128, 128], BF16, tag="ps_small")
            nc.tensor.transpose(
                pst[:, 0:BC], hid_sb[:, d * 128:(d + 1) * 128], ident_sb[0:BC, 0:BC]
            )
            nc.vector.tensor_copy(out=hT_sb[:, d, :], in_=pst[:, 0:BC])
        bias_sb = singles.tile([128, AT, BC], F32)
        for a in range(AT):
            psp = ps_proj.tile([128, BC], F32, tag="ps_proj")
            for d in range(DT):
                nc.tensor.matmul(
                    psp,
                    lhsT=w_dec_sb[:, d, a * 128:(a + 1) * 128],
                    rhs=hT_sb[:, d, :],
                    start=(d == 0),
                    stop=(d == DT - 1),
                )
            nc.vector.tensor_scalar_add(bias_sb[:, a, :], psp, b_sum_sb[:, a:a + 1])

        copy_engines = (
            lambda out, in_: nc.vector.tensor_copy(out=out, in_=in_),
            lambda out, in_: nc.scalar.copy(out=out, in_=in_),
        )

        # deferred alpha-transpose + context emission (software pipelining:
        # keeps pair g's softmax-dependent PE ops out of the way of pair g+1's
        # transpose/proj stream so the in-order PE queue never stalls)
        def emit_ctx(g, nats_g, alpha_bf_g):
            for bi, b in enumerate((2 * g, 2 * g + 1)):
                r0, r1 = nats_g[bi]
                co = bi * N
                aT = pool_aT.tile([128, 2], BF16, tag="at")
                pa0 = ps_small.tile([128, 128], BF16, tag="ps_small")
                nc.tensor.transpose(
                    pa0[:, 0:1], alpha_bf_g[0:1, co:co + 128], ident_sb[0:1, 0:1]
                )
                nc.vector.tensor_copy(out=aT[:, 0:1], in_=pa0[:, 0:1])
                pa1 = ps_small.tile([128, 128], BF16, tag="ps_small")
                nc.tensor.transpose(
                    pa1[0:N1, 0:1], alpha_bf_g[0:1, co + 128:co + N],
                    ident_sb[0:1, 0:1],
                )
                nc.vector.tensor_copy(out=aT[0:N1, 1:2], in_=pa1[0:N1, 0:1])

                ctx_sb = pool_ctx.tile([1, E], F32, tag="ctxs")
                for p in range(4):
                    sl = slice(p * 512, (p + 1) * 512)
                    psc = ps_ctx.tile([1, 512], F32, tag="ps_ctx")
                    nc.tensor.matmul(
                        psc, lhsT=aT[:, 0:1], rhs=r0[:, sl], start=True, stop=False
                    )
                    nc.tensor.matmul(
                        psc, lhsT=aT[0:N1, 1:2], rhs=r1[:, sl][0:N1, :],
                        start=False, stop=True,
                    )
                    copy_engines[p % 2](ctx_sb[0:1, sl], psc)
                nc.sync.dma_start(out=ctx_d[b, :], in_=ctx_sb[0:1, :])

        # xbar mode: manual ring of padded r1 tiles ([80, E]; rows 68:80
        # memset once -- xbar needs partition counts that are multiples of 16)
        r1_ring = []
        if USE_XBAR:
            for i in range(12):
                t = singles.tile([N1P, E], BF16, tag=f"r1ring{i}")
                nc.vector.memset(t[N1:N1P, :], 0)
                r1_ring.append(t)

        # ---- main loop over pairs ----
        pending = None
        for g in range(NPAIRS):
            b0, b1 = 2 * g, 2 * g + 1
            nats = []
            for b in (b0, b1):
                # SWDGE cast-DMA measured ~6x below line rate; land fp32 via
                # HWDGE at full rate and cast on the otherwise-idle GpSimd.
                r0f = pool_r0f.tile([128, E], F32, tag="r0f")
                nc.sync.dma_start(out=r0f, in_=enc_d[b * N:b * N + N0, :])
                r0 = pool_r0.tile([128, E], BF16, tag="r0")
                nc.gpsimd.tensor_copy(out=r0, in_=r0f)
                r1f = pool_r1f.tile([N1, E], F32, tag="r1f")
                nc.sync.dma_start(
                    out=r1f, in_=enc_d[b * N + N0:(b + 1) * N, :]
                )
                if USE_XBAR:
                    r1 = r1_ring[b % 12]
                else:
                    r1 = pool_r1.tile([N1, E], BF16, tag="r1")
                nc.vector.tensor_copy(out=r1[0:N1, :], in_=r1f)
                nats.append((r0, r1))

            # transpose to encT
            if USE_XBAR:
                # [128, ET, 2, 208]: DMA xbar transposes straight into SBUF.
                # One whole-row-tile DMA each: out[f, e, p] = in[p, 128e + f].
                encT = pool_encT.tile([128, ET, 2, NBP], BF16, tag="enct")
                for bi, (r0, r1) in enumerate(nats):
                    nc.sync.dma_start(
                        out=encT[:, :, bi, 0:128], in_=r0[:, :], transpose=True,
                    )
                    nc.sync.dma_start(
                        out=encT[:, :, bi, 128:NBP], in_=r1[:, :], transpose=True,
                    )
            else:
                # [128, ET, 392]; PE transposes -> one psum bank per e-chunk,
                # copied out with a single wide op
                encT = pool_encT.tile([128, ET, 2 * N], BF16, tag="enct")
                ci = 0
                for e in range(ET):
                    sl = slice(e * 128, (e + 1) * 128)
                    pt = ps_small.tile([128, 2 * N], BF16, tag="ps_small")
                    for bi, (r0, r1) in enumerate(nats):
                        co = bi * N
                        nc.tensor.matmul(
                            pt[:, co:co + 128], r0[:, sl], ident_sb,
                            is_transpose=True, skip_group_check=True,
                        )
                        nc.tensor.matmul(
                            pt[:, co + 128:co + N], r1[:, sl][0:N1, :],
                            ident_sb[0:N1, 0:N1],
                            is_transpose=True, skip_group_check=True,
                        )
                    copy_engines[ci % 2](encT[:, e, :], pt)
                    ci += 1

            # projection att1.T = W_enc.T @ enc.T  (+bias+relu on ScalarE)
            attT = pool_attT.tile([128, AT, 2 * N], BF16, tag="attt")
            for a in range(AT):
                psp = ps_proj.tile([128, 2 * N], F32, tag="ps_proj")
                for e in range(ET):
                    rhs = encT[:, e, :, 0:N] if USE_XBAR else encT[:, e, :]
                    nc.tensor.matmul(
                        psp,
                        lhsT=w_enc_sb[:, e, a * 128:(a + 1) * 128],
                        rhs=rhs,
                        start=(e == 0),
                        stop=(e == ET - 1),
                    )
                for bi, b in enumerate((b0, b1)):
                    nc.scalar.activation(
                        out=attT[:, a, bi * N:(bi + 1) * N],
                        in_=psp[:, bi * N:(bi + 1) * N],
                        func=AF.Relu,
                        bias=bias_sb[:, a, b:b + 1],
                        scale=1.0,
                    )

            # context of the PREVIOUS pair goes here in the PE stream: its
            # deps (softmax g-1) completed while this pair's proj ran.
            if pending is not None:
                emit_ctx(*pending)

            # scores = att.T . W_full  -> [1, 392]
            pss = ps_vec.tile([1, 2 * N], F32, tag="ps_vec")
            for a in range(AT):
                nc.tensor.matmul(
                    pss,
                    lhsT=w_full_sb[:, a:a + 1],
                    rhs=attT[:, a, :],
                    start=(a == 0),
                    stop=(a == AT - 1),
                )

            # softmax over each batch's 196 scores (on one partition)
            alpha_f = pool_al.tile([1, 2 * N], F32, tag="alph")
            alpha_bf = pool_al.tile([1, 2 * N], BF16, tag="alphb")
            stat = pool_al.tile([1, 4], F32, tag="stat")
            for bi in range(2):
                lo, hi = bi * N, (bi + 1) * N
                nc.vector.reduce_max(
                    out=stat[0:1, bi:bi + 1], in_=pss[0:1, lo:hi], axis=AX.X,
                    negate=True,
                )
                nc.scalar.activation(
                    out=alpha_f[0:1, lo:hi], in_=pss[0:1, lo:hi], func=AF.Exp,
                    bias=stat[0:1, bi:bi + 1], scale=1.0,
                    accum_out=stat[0:1, 2 + bi:3 + bi],
                )
                nc.vector.reciprocal(
                    out=stat[0:1, 2 + bi:3 + bi], in_=stat[0:1, 2 + bi:3 + bi]
                )
                nc.vector.tensor_scalar_mul(
                    alpha_f[0:1, lo:hi], alpha_f[0:1, lo:hi],
                    stat[0:1, 2 + bi:3 + bi],
                )
            nc.vector.tensor_copy(out=alpha_bf, in_=alpha_f)
            nc.sync.dma_start(
                out=alpha_d[2 * g * N:(2 * g + 2) * N], in_=alpha_f[0:1, :]
            )
            pending = (g, nats, alpha_bf)

        emit_ctx(*pending)

    nc.finalize()
    return nc


_CACHE = {}


def _get_nc():
    if "nc" not in _CACHE:
        _CACHE["nc"] = build_nc()
    return _CACHE["nc"]


def make_in_maps(encoder_out, hidden, W_enc, b_enc, W_dec, b_dec, W_full, b_full):
    bf = ml_dtypes.bfloat16
    w_enc_bf = np.ascontiguousarray(W_enc, np.float32).astype(bf)
    w_dec_bf = np.ascontiguousarray(W_dec, np.float32).astype(bf)
    w_full_t = np.ascontiguousarray(
        np.asarray(W_full, np.float32)[:, 0].reshape(AT, 128).T
    ).astype(bf)
    b_sum_t = np.ascontiguousarray(
        (np.asarray(b_enc, np.float32) + np.asarray(b_dec, np.float32))
        .reshape(AT, 128).T
    )
    ident = np.eye(128, dtype=bf)
    hidden_bf = np.asarray(hidden, np.float32).astype(bf)
    enc = np.asarray(encoder_out, np.float32)

    in_maps = []
    for c in range(NCORES):
        sl = slice(c * BC, (c + 1) * BC)
        in_maps.append({
            "enc": np.ascontiguousarray(enc[sl].reshape(BC * N, E)),
            "hidden_bf": np.ascontiguousarray(hidden_bf[sl]),
            "w_enc_bf": w_enc_bf,
            "w_dec_bf": w_dec_bf,
            "w_full_t": w_full_t,
            "b_sum_t": b_sum_t,
            "ident": ident,
        })
    return in_maps


def kernel(encoder_out, hidden, W_enc, b_enc, W_dec, b_dec, W_full, b_full):
    nc = _get_nc()
    in_maps = make_in_maps(
        encoder_out, hidden, W_enc, b_enc, W_dec, b_dec, W_full, b_full
    )
    res = run_bass_kernel_spmd(nc, in_maps, list(range(NCORES)))
    context = np.concatenate(
        [np.asarray(r["context"], np.float32) for r in res.results], axis=0
    )
    alpha = np.concatenate(
        [np.asarray(r["alpha"], np.float32).reshape(BC, N) for r in res.results],
        axis=0,
    )
    return context, alpha


if __name__ == "__main__":
    nc = _get_nc()
    print("built ok:", sum(len(b.instructions) for b in nc.main_func.blocks), "instructions")
